# Initial kernel scaffold
#
"""Multi-head attention (B=4, L=2048, d_model=1024, 16 heads) on 8 trn2 cores.

Sharding: core = (batch b, head-half hg). Each core computes 8 heads of one
batch element and a partial FC output [2048, 1024]; host sums the two
head-halves per batch and adds the residual.

Device layout (per core) is "transposed score" flash-style attention:
  qhT/khT [dh, seq] bf16, vh [seq, h, 65] bf16 (col 64 = ones)
  S^T[k, q] = khT.T-slice matmuls (K=64), exp+scale on ACT -> bf16,
  mask multiply on DVE (bf16 2x mode), PV with P^T stationary giving
  O[q, 65] with denominators in col 64, per-partition normalize, PE
  transpose to OT[hdv, q], FC matmul, DMA out.
"""

import numpy as np
import ml_dtypes

import concourse.bass as bass
import concourse.tile as tile
from concourse import mybir
from concourse.bass_utils import run_bass_kernel_spmd
from concourse.masks import make_identity

B, LQ, LK, DM = 4, 2048, 2048, 1024
NH, DK, DV = 16, 64, 64
H = 8            # heads per core
HD = H * DK      # 512
NCORES = 8

F32 = mybir.dt.float32
BF16 = mybir.dt.bfloat16
BF = ml_dtypes.bfloat16

AF = mybir.ActivationFunctionType

_CACHE = {}


def _build_nc():
    nc = bass.Bass()

    xq = nc.declare_dram_parameter("xqT", [DM, LQ], BF16, isOutput=False)
    xk = nc.declare_dram_parameter("xkT", [DM, LK], BF16, isOutput=False)
    xv = nc.declare_dram_parameter("xvT", [DM, LK], BF16, isOutput=False)
    mk = nc.declare_dram_parameter("maskT", [LK, LQ], BF16, isOutput=False)
    wq = nc.declare_dram_parameter("wq", [DM, HD], BF16, isOutput=False)
    wk = nc.declare_dram_parameter("wk", [DM, HD], BF16, isOutput=False)
    wv = nc.declare_dram_parameter("wv", [DM, HD], BF16, isOutput=False)
    wfc = nc.declare_dram_parameter("wfc", [HD, DM], BF16, isOutput=False)
    out = nc.declare_dram_parameter("out", [LQ, DM], F32, isOutput=True)

    with tile.TileContext(nc) as tc:
        _emit(nc, tc, xq, xk, xv, mk, wq, wk, wv, wfc, out)
    return nc


def _emit(nc, tc, xq, xk, xv, mk, wq, wk, wv, wfc, out):
    MT = DM // 128       # 8 m-tiles for projections
    KT = LK // 128       # 16 k tiles
    QH = 2               # q halves of 1024
    QHW = 1024           # q half width
    QB = 128             # q block (PV stationary M)
    NQB = QHW // QB      # 8 q blocks per half

    with (
        tc.tile_pool(name="const", bufs=1) as const,
        tc.tile_pool(name="wpool", bufs=1) as wpool,
        tc.tile_pool(name="xin", bufs=9) as xin,
        tc.tile_pool(name="act", bufs=1) as actp,
        tc.tile_pool(name="expp", bufs=3) as expp,
        tc.tile_pool(name="ptp", bufs=3) as ptp,
        tc.tile_pool(name="onp", bufs=4) as onp,
        tc.tile_pool(name="fout", bufs=2) as fout,
        tc.tile_pool(name="proj_ps", bufs=3, space="PSUM") as proj_ps,
        tc.tile_pool(name="s_ps", bufs=2, space="PSUM") as s_ps,
        tc.tile_pool(name="o_ps", bufs=2, space="PSUM") as o_ps,
        tc.tile_pool(name="t_ps", bufs=1, space="PSUM") as t_ps,
        tc.tile_pool(name="fc_ps", bufs=1, space="PSUM") as fc_ps,
    ):
        # ---- constants / big SBUF residents ----
        ident = const.tile([128, 128], BF16)
        make_identity(nc, ident)

        wq_sb = wpool.tile([128, MT, HD], BF16, tag="wq")
        wk_sb = wpool.tile([128, MT, HD], BF16, tag="wk")
        wv_sb = wpool.tile([128, MT, HD], BF16, tag="wv")
        nc.sync.dma_start(out=wq_sb, in_=wq[:].rearrange("(t p) d -> p t d", p=128))
        nc.sync.dma_start(out=wk_sb, in_=wk[:].rearrange("(t p) d -> p t d", p=128))
        nc.sync.dma_start(out=wv_sb, in_=wv[:].rearrange("(t p) d -> p t d", p=128))

        wfc_sb = const.tile([128, HD // 128, DM], BF16)
        nc.sync.dma_start(out=wfc_sb, in_=wfc[:].rearrange("(t p) d -> p t d", p=128))

        mask_sb = const.tile([128, KT, LQ], BF16)
        nc.sync.dma_start(out=mask_sb, in_=mk[:].rearrange("(t p) q -> p t q", p=128))

        qhT = const.tile([128, H // 2, LQ], BF16)   # [head-pair dh 128, j, q]
        khT = const.tile([128, H // 2, LK], BF16)
        vh = const.tile([128, KT, H, DV + 1], BF16)  # [k%128, kt, h, dv|one]
        OT = const.tile([128, H // 2, LQ], BF16)    # [head-pair dv 128, j, q]

        nc.vector.memset(vh[:, :, :, DV], 1.0)

        # ---- projections ----
        # khT/qhT: out^T form: lhsT = w[mt, dh-chunk], rhs = xT[mt, seq-chunk]
        def proj_T(w_sb, x_dram, dst, seq_chunks):
            for sc in seq_chunks:
                xts = []
                for mt in range(MT):
                    xt = xin.tile([128, 512], BF16, tag="x")
                    nc.sync.dma_start(
                        out=xt,
                        in_=x_dram[:].rearrange("(t p) s -> p t s", p=128)[
                            :, mt, sc * 512:(sc + 1) * 512
                        ],
                    )
                    xts.append(xt)
                for j in range(H // 2):
                    ps = proj_ps.tile([128, 512], F32, tag="pp")
                    for mt in range(MT):
                        nc.tensor.matmul(
                            ps,
                            lhsT=w_sb[:, mt, j * 128:(j + 1) * 128],
                            rhs=xts[mt],
                            start=(mt == 0),
                            stop=(mt == MT - 1),
                        )
                    nc.scalar.copy(dst[:, j, sc * 512:(sc + 1) * 512], ps)

        # vh: natural form: lhsT = xvT[mt, k-chunk 128], rhs = wv[mt, 512]
        def proj_v():
            for sc in range(4):
                xts = []
                for mt in range(MT):
                    xt = xin.tile([128, 512], BF16, tag="x")
                    nc.sync.dma_start(
                        out=xt,
                        in_=xv[:].rearrange("(t p) s -> p t s", p=128)[
                            :, mt, sc * 512:(sc + 1) * 512
                        ],
                    )
                    xts.append(xt)
                for kc in range(4):
                    kt = sc * 4 + kc
                    ps = proj_ps.tile([128, 512], F32, tag="pp")
                    for mt in range(MT):
                        nc.tensor.matmul(
                            ps,
                            lhsT=xts[mt][:, kc * 128:(kc + 1) * 128],
                            rhs=wv_sb[:, mt, :],
                            start=(mt == 0),
                            stop=(mt == MT - 1),
                        )
                    nc.vector.tensor_copy(
                        vh[:, kt, :, 0:DV].rearrange("p h d -> p (h d)"),
                        ps,
                    )

        proj_T(wk_sb, xk, khT, range(4))
        proj_T(wq_sb, xq, qhT, range(2))
        proj_v()
        proj_T(wq_sb, xq, qhT, range(2, 4))

        # ---- attention + FC per q-half ----
        for qh in range(QH):
            q0 = qh * QHW
            for j in range(H // 2):
                tps = t_ps.tile([128, NQB, QB], BF16, tag="tr")
                for r in range(2):
                    h = 2 * j + r
                    p0 = r * 64
                    oaug_lo = o_ps.tile([128, 4, DV + 1], F32, tag="oaug")
                    oaug_hi = o_ps.tile([128, 4, DV + 1], F32, tag="oaug")
                    oaugs = (oaug_lo, oaug_hi)
                    for kt in range(KT):
                        s = s_ps.tile([128, QHW], F32, tag="s")
                        for half in range(2):
                            nc.tensor.matmul(
                                s[:, half * 512:(half + 1) * 512],
                                lhsT=khT[p0:p0 + 64, j, kt * 128:(kt + 1) * 128],
                                rhs=qhT[p0:p0 + 64, j,
                                        q0 + half * 512:q0 + (half + 1) * 512],
                                start=True,
                                stop=True,
                            )
                        e = expp.tile([128, QHW], BF16, tag="e")
                        nc.scalar.activation(e, s, AF.Exp, scale=0.125)
                        p = ptp.tile([128, QHW], BF16, tag="p")
                        nc.vector.tensor_mul(p, e, mask_sb[:, kt, q0:q0 + QHW])
                        for qb in range(NQB):
                            nc.tensor.matmul(
                                oaugs[qb // 4][:, qb % 4, :],
                                lhsT=p[:, qb * QB:(qb + 1) * QB],
                                rhs=vh[:, kt, h, :],
                                start=(kt == 0),
                                stop=(kt == KT - 1),
                            )
                    rec = onp.tile([128, 4], F32, tag="rec")
                    for g in range(2):
                        nc.vector.reciprocal(rec[:, g * 2... ], oaugs[g][:, :, DV])
                    for qb in range(NQB):
                        on = onp.tile([128, DV], BF16, tag="on")
                        nc.vector.tensor_scalar_mul(
                            on,
                            oaugs[qb // 4][:, qb % 4, 0:DV],
                            rec[:, qb:qb + 1],
                        )
                        nc.tensor.transpose(
                            tps[p0:p0 + 64, qb, :], on, ident
                        )
                nc.vector.tensor_copy(
                    OT[:, j, q0:q0 + QHW],
                    tps.rearrange("p b q -> p (b q)"),
                )

            # FC for this q half
            for qb in range(NQB):
                fo = fout.tile([128, DM], F32, tag="fo")
                for half in range(2):
                    fc = fc_ps.tile([128, 512], F32, tag="fc")
                    for j in range(H // 2):
                        nc.tensor.matmul(
                            fc,
                            lhsT=OT[:, j, q0 + qb * QB:q0 + (qb + 1) * QB],
                            rhs=wfc_sb[:, j, half * 512:(half + 1) * 512],
                            start=(j == 0),
                            stop=(j == H // 2 - 1),
                        )
                    nc.vector.tensor_copy(fo[:, half * 512:(half + 1) * 512], fc)
                nc.sync.dma_start(
                    out=out[q0 + qb * QB:q0 + (qb + 1) * QB, :],
                    in_=fo,
                )


def kernel(q, k, v, mask, w_qs, w_ks, w_vs, w_fc):
    if "nc" not in _CACHE:
        _CACHE["nc"] = _build_nc()
    nc = _CACHE["nc"]

    q = np.asarray(q, np.float32)
    k = np.asarray(k, np.float32)
    v = np.asarray(v, np.float32)
    mask = np.asarray(mask)
    in_maps = []
    for core in range(NCORES):
        b, hg = core // 2, core % 2
        sl = slice(hg * HD, (hg + 1) * HD)
        in_maps.append({
            "xqT": np.ascontiguousarray(q[b].T).astype(BF),
            "xkT": np.ascontiguousarray(k[b].T).astype(BF),
            "xvT": np.ascontiguousarray(v[b].T).astype(BF),
            "maskT": np.ascontiguousarray(mask[b].T.astype(np.float32)).astype(BF),
            "wq": np.asarray(w_qs)[:, sl].astype(BF),
            "wk": np.asarray(w_ks)[:, sl].astype(BF),
            "wv": np.asarray(w_vs)[:, sl].astype(BF),
            "wfc": np.asarray(w_fc)[sl, :].astype(BF),
        })
    res = run_bass_kernel_spmd(nc, in_maps, list(range(NCORES)))
    outs = [res.results[i]["out"] for i in range(NCORES)]
    full = np.stack([outs[2 * b] + outs[2 * b + 1] for b in range(B)])
    return (full + q).astype(np.float32)


# revision 23
# speedup vs baseline: 1.1702x; 1.1702x over previous
"""Multi-head attention (B=4, L=2048, d_model=1024, 16 heads) on 8 trn2 cores.

Sharding: core = (batch b, head-half hg). Each core computes 8 heads of one
batch element and a partial FC output [2048, 1024]; host sums the two
head-halves per batch and adds the residual.

Device layout (per core) is "transposed score" flash-style attention:
  qhT/khT [dh, seq] bf16, vh [seq, h, 65] bf16 (col 64 = ones)
  S^T[k, q] = khT.T-slice matmuls (K=64), exp+scale on ACT -> bf16,
  mask multiply on DVE (bf16 2x mode), PV with P^T stationary giving
  O[q, 65] with denominators in col 64, per-partition normalize, PE
  transpose to OT[hdv, q], FC matmul, DMA out.
"""

import numpy as np
import ml_dtypes

import concourse.bass as bass
import concourse.tile as tile
from concourse import bacc, mybir
from concourse.bass_utils import run_bass_kernel_spmd
from concourse.masks import make_identity

B, LQ, LK, DM = 4, 2048, 2048, 1024
NH, DK, DV = 16, 64, 64
H = 8            # heads per core
HD = H * DK      # 512
NCORES = 8

F32 = mybir.dt.float32
BF16 = mybir.dt.bfloat16
BF = ml_dtypes.bfloat16

AF = mybir.ActivationFunctionType

_CACHE = {}


def _build_nc():
    nc = bacc.Bacc()

    xq = nc.declare_dram_parameter("xqT", [DM, LQ], BF16, isOutput=False)
    xk = nc.declare_dram_parameter("xkT", [DM, LK], BF16, isOutput=False)
    xv = nc.declare_dram_parameter("xvT", [DM, LK], BF16, isOutput=False)
    mk = nc.declare_dram_parameter("maskT", [LK, LQ], BF16, isOutput=False)
    wq = nc.declare_dram_parameter("wq", [DM, HD], BF16, isOutput=False)
    wk = nc.declare_dram_parameter("wk", [DM, HD], BF16, isOutput=False)
    wv = nc.declare_dram_parameter("wv", [DM, HD], BF16, isOutput=False)
    wfc = nc.declare_dram_parameter("wfc", [HD, DM], BF16, isOutput=False)
    out = nc.declare_dram_parameter("out", [LQ, DM], F32, isOutput=True)

    with tile.TileContext(nc) as tc:
        _emit(nc, tc, xq, xk, xv, mk, wq, wk, wv, wfc, out)
    nc.compile()
    return nc


def _emit(nc, tc, xq, xk, xv, mk, wq, wk, wv, wfc, out):
    MT = DM // 128       # 8 m-tiles for projections
    KT = LK // 128       # 16 k tiles
    QH = 2               # q halves of 1024
    QHW = 1024           # q half width
    QB = 128             # q block (PV stationary M)
    NQB = QHW // QB      # 8 q blocks per half

    with tc.tile_pool(name="const", bufs=1) as const:
        # ---- constants / big SBUF residents ----
        ident = const.tile([128, 128], BF16)
        make_identity(nc, ident)

        # tiles declared here; DMAs for mask/wfc are emitted mid-projection
        # so they don't contend with the critical-path prefix loads
        wfc_sb = const.tile([128, HD // 128, DM], BF16)
        mask_sb = const.tile([128, KT, LQ], BF16)

        qhT = const.tile([128, H // 2, LQ], BF16)   # [head-pair dh 128, j, q]
        khT = const.tile([128, H // 2, LK], BF16)
        vh = const.tile([128, KT, H, DV + 1], BF16)  # [k%128, kt, h, dv|one]
        OT = const.tile([128, H // 2, LQ], BF16)    # [head-pair dv 128, j, q]

        nc.vector.memset(vh[:, :, :, DV], 1.0)

        # ---- projections (own scope: frees PSUM banks + weight SBUF) ----
        with (
            tc.tile_pool(name="wpool", bufs=1) as wpool,
            tc.tile_pool(name="xin", bufs=9) as xin,
            tc.tile_pool(name="proj_ps", bufs=3, space="PSUM") as proj_ps,
        ):
            wq_sb = wpool.tile([128, MT, HD], BF16, tag="wq")
            wk_sb = wpool.tile([128, MT, HD], BF16, tag="wk")
            wv_sb = wpool.tile([128, MT, HD], BF16, tag="wv")
            for sb, dr in ((wq_sb, wq), (wk_sb, wk), (wv_sb, wv)):
                for t in range(MT):
                    nc.sync.dma_start(
                        out=sb[:, t, :], in_=dr[t * 128:(t + 1) * 128, :]
                    )

            # khT/qhT: out^T form: lhsT = w[mt, dh-chunk], rhs = xT[mt, chunk]
            def proj_T(w_sb, x_dram, dst, seq_chunks):
                for sc in seq_chunks:
                    xts = []
                    for mt in range(MT):
                        xt = xin.tile([128, 512], BF16, tag="x")
                        nc.sync.dma_start(
                            out=xt,
                            in_=x_dram[:].rearrange("(t p) s -> p t s", p=128)[
                                :, mt, sc * 512:(sc + 1) * 512
                            ],
                        )
                        xts.append(xt)
                    for j in range(H // 2):
                        ps = proj_ps.tile([128, 512], F32, tag="pp")
                        for mt in range(MT):
                            nc.tensor.matmul(
                                ps,
                                lhsT=w_sb[:, mt, j * 128:(j + 1) * 128],
                                rhs=xts[mt],
                                start=(mt == 0),
                                stop=(mt == MT - 1),
                            )
                        nc.scalar.copy(dst[:, j, sc * 512:(sc + 1) * 512], ps)

            # vh: natural form: lhsT = xvT[mt, k-chunk 128], rhs = wv[mt, 512]
            def proj_v():
                for sc in range(4):
                    xts = []
                    for mt in range(MT):
                        xt = xin.tile([128, 512], BF16, tag="x")
                        nc.sync.dma_start(
                            out=xt,
                            in_=xv[:].rearrange("(t p) s -> p t s", p=128)[
                                :, mt, sc * 512:(sc + 1) * 512
                            ],
                        )
                        xts.append(xt)
                    for kc in range(4):
                        kt = sc * 4 + kc
                        ps = proj_ps.tile([128, 512], F32, tag="pp")
                        for mt in range(MT):
                            nc.tensor.matmul(
                                ps,
                                lhsT=xts[mt][:, kc * 128:(kc + 1) * 128],
                                rhs=wv_sb[:, mt, :],
                                start=(mt == 0),
                                stop=(mt == MT - 1),
                            )
                        nc.vector.tensor_copy(
                            vh[:, kt, :, 0:DV],
                            ps.rearrange("p (h d) -> p h d", h=H),
                        )

            proj_T(wk_sb, xk, khT, range(4))
            proj_T(wq_sb, xq, qhT, range(2))
            for t in range(KT):
                nc.gpsimd.dma_start(
                    out=mask_sb[:, t, :], in_=mk[t * 128:(t + 1) * 128, :]
                )
            for t in range(HD // 128):
                nc.sync.dma_start(
                    out=wfc_sb[:, t, :], in_=wfc[t * 128:(t + 1) * 128, :]
                )
            proj_v()
            proj_T(wq_sb, xq, qhT, range(2, 4))

        # ---- attention per q-half ----
        with (
            tc.tile_pool(name="expp", bufs=3) as expp,
            tc.tile_pool(name="ptp", bufs=3) as ptp,
            tc.tile_pool(name="otu", bufs=2) as otu,
            tc.tile_pool(name="onp", bufs=4) as onp,
            tc.tile_pool(name="fout", bufs=3) as fout,
            tc.tile_pool(name="s_ps", bufs=2, space="PSUM") as s_ps,
            tc.tile_pool(name="oT_ps", bufs=1, space="PSUM") as oT_ps,
            tc.tile_pool(name="sm_ps", bufs=2, space="PSUM") as sm_ps,
        ):
          for qh in range(QH):
            q0 = qh * QHW
            for j in range(H // 2):
                tps = sm_ps.tile([128, NQB, QB], BF16, tag="sm")
                for r in range(2):
                    h = 2 * j + r
                    p0 = r * 64
                    oT = oT_ps.tile([DV + 1, QHW], F32, tag="oT")
                    for kt in range(KT):
                        s = s_ps.tile([128, QHW], F32, tag="s")
                        for half in range(2):
                            nc.tensor.matmul(
                                s[:, half * 512:(half + 1) * 512],
                                lhsT=khT[p0:p0 + 64, j, kt * 128:(kt + 1) * 128],
                                rhs=qhT[p0:p0 + 64, j,
                                        q0 + half * 512:q0 + (half + 1) * 512],
                                start=True,
                                stop=True,
                            )
                        e = expp.tile([128, QHW], BF16, tag="e")
                        nc.scalar.activation(e, s, AF.Exp, scale=0.125)
                        p = ptp.tile([128, QHW], BF16, tag="p")
                        nc.vector.tensor_mul(p, e, mask_sb[:, kt, q0:q0 + QHW])
                        for half in range(2):
                            nc.tensor.matmul(
                                oT[:, half * 512:(half + 1) * 512],
                                lhsT=vh[:, kt, h, :],
                                rhs=p[:, half * 512:(half + 1) * 512],
                                start=(kt == 0),
                                stop=(kt == KT - 1),
                            )
                    # unnormalized O^T (+den row 64) -> SBUF bf16
                    ou = otu.tile([DV + 1, QHW], BF16, tag="ou")
                    nc.vector.tensor_copy(ou, oT)
                    # transpose to q-partition layout [q, dv|den]
                    oq = sm_ps.tile([128, NQB, DV + 2], BF16, tag="sm")
                    for qb in range(NQB):
                        nc.tensor.transpose(
                            oq[:, qb, 0:DV + 1],
                            ou[:, qb * QB:(qb + 1) * QB],
                            ident[0:DV + 1, 0:DV + 1],
                        )
                    rec = onp.tile([128, NQB], F32, tag="rec")
                    nc.vector.reciprocal(rec, oq[:, :, DV])
                    for qb in range(NQB):
                        on = onp.tile([128, DV], BF16, tag="on")
                        nc.vector.tensor_scalar_mul(
                            on, oq[:, qb, 0:DV], rec[:, qb:qb + 1]
                        )
                        nc.tensor.transpose(
                            tps[p0:p0 + 64, qb, :], on, ident
                        )
                nc.vector.tensor_copy(
                    OT[:, j, q0:q0 + QHW],
                    tps.rearrange("p b q -> p (b q)"),
                )

            # FC for this q half, interleaved (shares sm_ps bank slots)
            for qb in range(NQB):
                fo = fout.tile([128, DM], F32, tag="fo")
                for half in range(2):
                    fc = sm_ps.tile([128, 512], F32, tag="sm")
                    for j in range(H // 2):
                        nc.tensor.matmul(
                            fc,
                            lhsT=OT[:, j, q0 + qb * QB:q0 + (qb + 1) * QB],
                            rhs=wfc_sb[:, j, half * 512:(half + 1) * 512],
                            start=(j == 0),
                            stop=(j == H // 2 - 1),
                        )
                    nc.vector.tensor_copy(
                        fo[:, half * 512:(half + 1) * 512], fc
                    )
                nc.sync.dma_start(
                    out=out[q0 + qb * QB:q0 + (qb + 1) * QB, :],
                    in_=fo,
                )


def get_nc():
    if "nc" not in _CACHE:
        _CACHE["nc"] = _build_nc()
    return _CACHE["nc"]


def make_in_maps(q, k, v, mask, w_qs, w_ks, w_vs, w_fc):
    q = np.asarray(q, np.float32)
    k = np.asarray(k, np.float32)
    v = np.asarray(v, np.float32)
    mask = np.asarray(mask)
    in_maps = []
    for core in range(NCORES):
        b, hg = core // 2, core % 2
        sl = slice(hg * HD, (hg + 1) * HD)
        in_maps.append({
            "xqT": np.ascontiguousarray(q[b].T).astype(BF),
            "xkT": np.ascontiguousarray(k[b].T).astype(BF),
            "xvT": np.ascontiguousarray(v[b].T).astype(BF),
            "maskT": np.ascontiguousarray(mask[b].T.astype(np.float32)).astype(BF),
            "wq": np.asarray(w_qs)[:, sl].astype(BF),
            "wk": np.asarray(w_ks)[:, sl].astype(BF),
            "wv": np.asarray(w_vs)[:, sl].astype(BF),
            "wfc": np.asarray(w_fc)[sl, :].astype(BF),
        })
    return in_maps


def finish(results, q):
    q = np.asarray(q, np.float32)
    outs = [results[i]["out"] for i in range(NCORES)]
    full = np.stack([outs[2 * b] + outs[2 * b + 1] for b in range(B)])
    return (full + q).astype(np.float32)


def kernel(q, k, v, mask, w_qs, w_ks, w_vs, w_fc):
    nc = get_nc()
    in_maps = make_in_maps(q, k, v, mask, w_qs, w_ks, w_vs, w_fc)
    res = run_bass_kernel_spmd(nc, in_maps, list(range(NCORES)))
    return finish(res.results, q)
